# revision 1
# baseline (speedup 1.0000x reference)
"""LIIF-style super-resolution kernel for 8 Trainium2 NeuronCores.

Strategy:
  - Layer-0 of the MLP is linear in the gathered (unfolded) features, so it is
    precomputed once over the whole 128x128 feature grid as 10 accumulated
    matmuls per 128-position chunk (9 shifted-view matmuls for the 64 dense
    channels + 1 matmul covering the mask channel's 9 shifts and the bias row).
    Result: a 16384 x 256 fp16 table resident in SBUF.
  - Queries (65536 x 4 shifts) are sharded across the 8 cores (8192 each).
    Each core dma_gathers 256-float rows from its SBUF table; the transpose
    gather lands feature-major, exactly the layout the MLP matmuls need.
  - The per-query rel-coordinate contribution enters via a K=2 matmul into
    PSUM; a vector-engine tensor_tensor add fuses it with the gathered rows.
  - Layers 1-3 (256x256) run as 4 fp16 matmuls each per 512-query block with
    dual-op (bias+relu) tensor_scalar evacuations alternating between the
    vector and scalar engines. Layer 4 (256->3) is 2 matmuls.
  - The 4 shift predictions are blended on-device with host-computed
    local-ensemble area weights.
"""

import os
import numpy as np

N_CORES = 8
H = W = 128          # feature grid
HP = 130             # padded
Q_TOTAL = 65536
Q_CORE = Q_TOTAL // N_CORES          # 8192
QB = 512                             # query block (matmul free dim)
NQB = Q_CORE // QB                   # 16 blocks per core
NS = 4                               # shifts
D0 = 256                             # layer-0 width
IN_DIM = 587
SHIFTS = [(-1, -1), (-1, 1), (1, -1), (1, 1)]

_CACHE = {}


def _build_program(cks):
    import concourse.mybir as mybir
    import concourse.tile as tile
    from concourse import bacc

    f16 = mybir.dt.float16
    f32 = mybir.dt.float32
    i16 = mybir.dt.int16

    GQ = int(os.environ.get("BASS_KERNEL_GQ", "4"))
    nc = bacc.Bacc("TRN2", target_bir_lowering=False, debug=False,
                   num_devices=N_CORES, num_swdge_queues=GQ)

    featq_d = nc.dram_tensor("featq", [128, HP * HP], f16, kind="ExternalInput")
    maskq_d = nc.dram_tensor("maskq", [10, HP * HP], f16, kind="ExternalInput")
    w0gp_d = nc.dram_tensor("w0gp", [128, 3 * D0], f16, kind="ExternalInput")
    w0gs_d = nc.dram_tensor("w0gs", [64, 3 * D0], f16, kind="ExternalInput")
    wx_d = nc.dram_tensor("wx", [10, D0], f16, kind="ExternalInput")
    wq_d = nc.dram_tensor("wq", [2, D0], f16, kind="ExternalInput")
    wl_d = nc.dram_tensor("wl", [128, 12 * 128], f16, kind="ExternalInput")
    w4_d = nc.dram_tensor("w4", [128, 6], f16, kind="ExternalInput")
    bl_d = nc.dram_tensor("bl", [128, 7], f32, kind="ExternalInput")
    idx_d = nc.dram_tensor("idx", [128, NQB * 128], i16, kind="ExternalInput")
    rels_d = nc.dram_tensor("rels", [NQB * NS, 2, QB], f16, kind="ExternalInput")
    wb_d = nc.dram_tensor("wb", [12, Q_CORE], f16, kind="ExternalInput")
    out_d = nc.dram_tensor("out", [3, Q_CORE], f32, kind="ExternalOutput")

    with tile.TileContext(nc) as tc:
        with tc.tile_pool(name="constp", bufs=1) as constp, \
             tc.tile_pool(name="ztabdp", bufs=1, space="DRAM") as ztabdp:
            wl = constp.tile([128, 12 * 128], f16)
            nc.sync.dma_start(out=wl[:], in_=wl_d[:])
            w4 = constp.tile([128, 6], f16)
            nc.sync.dma_start(out=w4[:], in_=w4_d[:])
            wq = constp.tile([2, D0], f16)
            nc.sync.dma_start(out=wq[:], in_=wq_d[:])
            bl = constp.tile([128, 7], f32)
            nc.sync.dma_start(out=bl[:], in_=bl_d[:])
            idxs = constp.tile([128, NQB * 128], i16)
            nc.sync.dma_start(out=idxs[:], in_=idx_d[:])
            pred_d0 = None  # placeholder
            pred_d = ztabdp.tile([NS, 3, Q_CORE], f16, name="predd",
                                 space="DRAM")

            # ---------------- grid phase: build layer-0 table ----------------
            # featq rows 0-63: padded channels; rows 64-127: same shifted +1
            # element, so a K=128 matmul covers two 3x3 offsets at once.
            ztab_stack = tc.tile_pool(name="ztabp", bufs=1)
            ztabp = ztab_stack.__enter__()
            ztab = ztabp.tile([128, H, D0], f16, name="ztab")
            # dummy gather: forces the gpsimd 'mlp' ucode library load to
            # happen NOW (idle), not between grid and MLP phases where the
            # reload's quiesce barrier would serialize the whole kernel.
            dzi = constp.tile([128, 8], i16, name="dzi")
            nc.vector.memset(dzi[:], 0)
            nc.vector.memset(ztab[:, 0:1, :], 0)
            dzo = constp.tile([128, 2, 128], f16, name="dzo")
            nc.gpsimd.dma_gather(
                dzo[:], ztab[:, 0:1, :], dzi[:],
                num_idxs=128, num_idxs_reg=128, elem_size=D0, transpose=True,
                sbuf_tokens_per_rank=128, sbuf_free_dim_per_rank=D0 * 2,
                single_packet=False, queue_num=0)
            psg_stack = tc.tile_pool(name="psg", bufs=2, space="PSUM")
            psg = psg_stack.__enter__()
            gridp_stack = tc.tile_pool(name="gridp", bufs=1)
            gridp = gridp_stack.__enter__()
            if True:
                featq = gridp.tile([128, HP * HP], f16)
                nc.sync.dma_start(out=featq[:], in_=featq_d[:])
                maskq = gridp.tile([10, HP * HP], f16)
                nc.sync.dma_start(out=maskq[:], in_=maskq_d[:])
                w0gp = gridp.tile([128, 3 * D0], f16)
                nc.sync.dma_start(out=w0gp[:], in_=w0gp_d[:])
                w0gs = gridp.tile([64, 3 * D0], f16)
                nc.sync.dma_start(out=w0gs[:], in_=w0gs_d[:])
                wx = gridp.tile([10, D0], f16)
                nc.sync.dma_start(out=wx[:], in_=wx_d[:])

                for i in range(H):
                    ps = psg.tile([128, D0], f32, tag="psg")
                    for p, off in enumerate((0, HP, 2 * HP)):  # offset pairs
                        nc.tensor.matmul(
                            ps[:], featq[:, i * HP + off:i * HP + off + 128],
                            w0gp[:, p * D0:(p + 1) * D0],
                            start=(p == 0), stop=False)
                    for s, off in enumerate((2, HP + 2, 2 * HP + 2)):
                        nc.tensor.matmul(
                            ps[:], featq[0:64, i * HP + off:i * HP + off + 128],
                            w0gs[:, s * D0:(s + 1) * D0],
                            start=False, stop=False)
                    nc.tensor.matmul(
                        ps[:], maskq[:, i * HP:i * HP + 128], wx[:],
                        start=False, stop=True)
                    dst = ztab[:, i, :]
                    if i % 4 == 3:
                        nc.scalar.copy(dst, ps[:])
                    else:
                        nc.vector.tensor_copy(dst, ps[:])


            # ---------------- MLP phase ----------------
            with tc.tile_pool(name="zgp", bufs=3) as zgp, \
                 tc.tile_pool(name="relp", bufs=4) as relp, \
                 tc.tile_pool(name="actp", bufs=4) as actp, \
                 tc.tile_pool(name="hp", bufs=6) as hp, \
                 tc.tile_pool(name="psr", bufs=2, space="PSUM") as psr, \
                 tc.tile_pool(name="psl", bufs=4, space="PSUM") as psl:

                ev_ctr = [0]

                def evac_relu(dst, ps, bias_ap):
                    ev_ctr[0] += 1
                    if ev_ctr[0] % 2 == 0:
                        nc.vector.tensor_scalar(
                            out=dst, in0=ps, scalar1=bias_ap, scalar2=0.0,
                            op0=mybir.AluOpType.add, op1=mybir.AluOpType.max)
                    else:
                        nc.scalar.activation(
                            dst, ps, mybir.ActivationFunctionType.Relu,
                            bias=bias_ap)

                def evac_add(dst, ps, bias_ap):
                    ev_ctr[0] += 1
                    if ev_ctr[0] % 2 == 0:
                        nc.vector.tensor_scalar_add(dst, ps, bias_ap)
                    else:
                        nc.scalar.add(dst, ps, bias_ap)

                for qb in range(NQB):
                    zg = zgp.tile([128, 2, NS * QB], f16, tag="zg")
                    ck = cks[qb]
                    nc.gpsimd.dma_gather(
                        zg[:], ztab[:, 0:ck, :],
                        idxs[:, qb * 128:(qb + 1) * 128],
                        num_idxs=NS * QB, num_idxs_reg=NS * QB,
                        elem_size=D0, transpose=True,
                        sbuf_tokens_per_rank=128,
                        sbuf_free_dim_per_rank=D0 * 2,
                        single_packet=False, queue_num=(qb + 1) % GQ)
                    for s in range(NS):
                        rel = relp.tile([2, QB], f16, tag="rel")
                        nc.sync.dma_start(
                            out=rel[:], in_=rels_d[qb * NS + s, :, :])
                        # layer 0: rel-term matmul + gathered table rows
                        x0r = []
                        for c in range(2):
                            rp = psr.tile([128, QB], f32, tag="rp")
                            nc.tensor.matmul(
                                rp[:], wq[:, c * 128:(c + 1) * 128], rel[:],
                                start=True, stop=True)
                            x0 = actp.tile([128, QB], f16, tag="x0")
                            nc.vector.tensor_tensor(
                                out=x0[:], in0=rp[:],
                                in1=zg[:, c, s * QB:(s + 1) * QB],
                                op=mybir.AluOpType.add)
                            xr = actp.tile([128, QB], f16, tag="xr")
                            nc.vector.tensor_scalar_max(xr[:], x0[:], 0.0)
                            x0r.append(xr)
                        # layers 1..3
                        h = x0r
                        for l in range(3):
                            hn = []
                            for m in range(2):
                                ps = psl.tile([128, QB], f32, tag="pl")
                                for k in range(2):
                                    blk = (l * 4 + k * 2 + m) * 128
                                    nc.tensor.matmul(
                                        ps[:], wl[:, blk:blk + 128], h[k][:],
                                        start=(k == 0), stop=(k == 1))
                                hh = hp.tile([128, QB], f16, tag="h")
                                evac_relu(hh[:], ps[:],
                                          bl[:, 2 * l + m:2 * l + m + 1])
                                hn.append(hh)
                            h = hn
                        # layer 4
                        p4 = psl.tile([3, QB], f32, tag="pl")
                        for k in range(2):
                            nc.tensor.matmul(
                                p4[:], w4[:, k * 3:(k + 1) * 3], h[k][:],
                                start=(k == 0), stop=(k == 1))
                        p4s = hp.tile([3, QB], f16, tag="p4s", bufs=4)
                        evac_add(p4s[:], p4[:], bl[0:3, 6:7])
                        nc.sync.dma_start(
                            out=pred_d[s, :, qb * QB:(qb + 1) * QB],
                            in_=p4s[:])

            gridp_stack.__exit__(None, None, None)
            psg_stack.__exit__(None, None, None)
            ztab_stack.__exit__(None, None, None)

            # ---------------- blend ----------------
            with tc.tile_pool(name="blendp", bufs=1) as blendp:
                preds = []
                for s in range(NS):
                    pt = blendp.tile([3, Q_CORE], f16, tag=f"pr{s}",
                                     name=f"pr{s}")
                    nc.sync.dma_start(out=pt[:], in_=pred_d[s, :, :])
                    preds.append(pt)
                for s in range(NS):
                    wbs = blendp.tile([3, Q_CORE], f16, tag="wbs", name="wbs")
                    nc.sync.dma_start(out=wbs[:],
                                      in_=wb_d[s * 3:(s + 1) * 3, :])
                    nc.vector.tensor_mul(preds[s][:], preds[s][:], wbs[:])
                nc.vector.tensor_add(preds[0][:], preds[0][:], preds[1][:])
                nc.vector.tensor_add(preds[2][:], preds[2][:], preds[3][:])
                outt = blendp.tile([3, Q_CORE], f32)
                nc.vector.tensor_add(outt[:], preds[0][:], preds[2][:])
                nc.sync.dma_start(out=out_d[:], in_=outt[:])

    nc.compile()
    return nc


def _host_prep(masked_feat, mask, hr_coord, Ws, bs):
    f32 = np.float32
    # padded feature image: rows 0-63 channels, rows 64-127 shifted +1 elem
    featq = np.zeros((128, HP * HP), dtype=np.float16)
    ch = np.zeros((64, HP, HP), dtype=f32)
    ch[:, 1:129, 1:129] = masked_feat[0]
    chf = ch.reshape(64, -1).astype(np.float16)
    featq[0:64] = chf
    featq[64:128, 0:HP * HP - 1] = chf[:, 1:]
    # maskq: 9 pre-shifted mask rows + ones row (bias)
    maskq = np.zeros((10, HP * HP), dtype=np.float16)
    mpad = np.zeros((HP, HP), dtype=f32)
    mpad[1:129, 1:129] = mask[0, 0]
    mflat = mpad.reshape(-1)
    for o in range(9):
        ki, kj = o // 3, o % 3
        off = ki * HP + kj
        maskq[o, 0:HP * HP - off] = mflat[off:].astype(np.float16)
    maskq[9, :] = 1.0

    W0 = Ws[0].astype(f32)
    cc = np.arange(64) * 9
    # offset pairs (0,1),(130,131),(260,261) -> o pairs (0,1),(3,4),(6,7)
    w0gp = np.zeros((128, 3 * D0), dtype=np.float16)
    for p, (oa, ob) in enumerate(((0, 1), (3, 4), (6, 7))):
        w0gp[0:64, p * D0:(p + 1) * D0] = W0[cc + oa, :]
        w0gp[64:128, p * D0:(p + 1) * D0] = W0[cc + ob, :]
    # single offsets 2,132,262 -> o 2,5,8
    w0gs = np.zeros((64, 3 * D0), dtype=np.float16)
    for s, o in enumerate((2, 5, 8)):
        w0gs[:, s * D0:(s + 1) * D0] = W0[cc + o, :]
    wx = np.zeros((10, D0), dtype=np.float16)
    for o in range(9):
        wx[o] = W0[64 * 9 + o, :]               # mask channel weights
    wx[9] = bs[0].astype(f32)                   # bias row
    wq = W0[585:587, :].astype(np.float16)  # rel rows (rel already scaled)

    wl = np.zeros((128, 12 * 128), dtype=np.float16)
    for l in range(3):
        Wl = Ws[l + 1].astype(f32)
        for k in range(2):
            for m in range(2):
                blk = (l * 4 + k * 2 + m) * 128
                wl[:, blk:blk + 128] = Wl[k * 128:(k + 1) * 128,
                                          m * 128:(m + 1) * 128]
    w4 = np.zeros((128, 6), dtype=np.float16)
    w4[:, 0:3] = Ws[4][0:128].astype(f32)
    w4[:, 3:6] = Ws[4][128:256].astype(f32)
    bl = np.zeros((128, 7), dtype=f32)
    for l in range(3):
        for m in range(2):
            bl[:, 2 * l + m] = bs[l + 1][m * 128:(m + 1) * 128]
    bl[0:3, 6] = bs[4]

    # per-query index / rel / area computation (mirrors reference in fp32)
    coord = hr_coord[0].astype(f32)            # [Q, 2]
    rx = f32(1.0 / H)
    eps = f32(1e-6)
    lo, hi = f32(-1 + 1e-6), f32(1 - 1e-6)
    idx_s, rel_s, area_s = [], [], []
    for (vx, vy) in SHIFTS:
        cy = np.clip(coord[:, 0] + f32(vx) * rx + eps, lo, hi)
        cx = np.clip(coord[:, 1] + f32(vy) * rx + eps, lo, hi)
        iy = np.clip(np.rint(((cy + f32(1.0)) * f32(H) - f32(1.0)) * f32(0.5)
                             ).astype(np.int32), 0, H - 1)
        ix = np.clip(np.rint(((cx + f32(1.0)) * f32(W) - f32(1.0)) * f32(0.5)
                             ).astype(np.int32), 0, W - 1)
        idx_s.append((iy * W + ix).astype(np.int16))
        qy = f32(-1.0) + (f32(2.0) * iy.astype(f32) + f32(1.0)) / f32(H)
        qx = f32(-1.0) + (f32(2.0) * ix.astype(f32) + f32(1.0)) / f32(W)
        r0 = (coord[:, 0] - qy) * f32(H)
        r1 = (coord[:, 1] - qx) * f32(W)
        rel_s.append(np.stack([r0, r1]))       # [2, Q]
        area_s.append(np.abs(r0 * r1) + f32(1e-9))
    tot = area_s[0] + area_s[1] + area_s[2] + area_s[3]
    wgt = [area_s[3 - s] / tot for s in range(NS)]   # local-ensemble swap

    idx_all = np.stack(idx_s)                 # [NS, Q]
    in_maps, orders = [], []
    cks = np.zeros((N_CORES, NQB), dtype=np.int64)
    for c in range(N_CORES):
        q0 = c * Q_CORE
        # sort this core's queries by the highest table row they touch, so
        # gather k only depends on a prefix of the table (grid/MLP overlap)
        key = idx_all[:, q0:q0 + Q_CORE].max(axis=0)
        order = np.argsort(key, kind="stable")
        orders.append(order)
        gidx = order + q0                      # global query ids, sorted
        idxw = np.zeros((128, NQB * 128), dtype=np.int16)
        rels = np.zeros((NQB * NS, 2, QB), dtype=np.float16)
        wb = np.zeros((12, Q_CORE), dtype=np.float16)
        for qb in range(NQB):
            gsl = gidx[qb * QB:(qb + 1) * QB]
            block = np.concatenate([idx_s[s][gsl] for s in range(NS)])
            cks[c, qb] = block.max() // 128 + 1
            idxw[:, qb * 128:(qb + 1) * 128] = np.tile(
                block.reshape(128, 16).T, (8, 1))
            for s in range(NS):
                rels[qb * NS + s] = rel_s[s][:, gsl].astype(np.float16)
        for s in range(NS):
            wb[s * 3:(s + 1) * 3, :] = wgt[s][gidx].astype(
                np.float16)[None, :]
        in_maps.append({
            "featq": featq, "maskq": maskq, "w0gp": w0gp, "w0gs": w0gs,
            "wx": wx, "wq": wq, "wl": wl, "w4": w4, "bl": bl,
            "idx": idxw, "rels": rels, "wb": wb,
        })
    cks_max = tuple(int(v) for v in cks.max(axis=0))
    return in_maps, orders, cks_max


def kernel(masked_feat, mask, gt_feat, hr_coord,
           W0, b0, W1, b1, W2, b2, W3, b3, W4, b4):
    from concourse.bass_utils import run_bass_kernel_spmd

    Ws = [W0, W1, W2, W3, W4]
    bs = [b0, b1, b2, b3, b4]
    in_maps, orders, cks = _host_prep(
        np.asarray(masked_feat), np.asarray(mask), np.asarray(hr_coord),
        [np.asarray(w) for w in Ws], [np.asarray(b) for b in bs])
    if int(os.environ.get("BASS_KERNEL_FULLTAB", "0")):
        cks = tuple([H] * NQB)
    if _CACHE.get("cks") != cks:
        _CACHE["nc"] = _build_program(cks)
        _CACHE["cks"] = cks
    nc = _CACHE["nc"]

    trace = bool(int(os.environ.get("BASS_KERNEL_TRACE", "0")))
    res = run_bass_kernel_spmd(nc, in_maps, list(range(N_CORES)), trace=trace)
    _CACHE["last_results"] = res

    full = np.zeros((3, Q_TOTAL), dtype=np.float32)
    for c in range(N_CORES):
        full[:, c * Q_CORE + orders[c]] = res.results[c]["out"]
    return full.reshape(1, 3, 256, 256).astype(np.float32)



# revision 7
# speedup vs baseline: 2.2013x; 2.2013x over previous
"""LIIF-style super-resolution kernel for 8 Trainium2 NeuronCores.

Strategy (v2):
  - Queries are sorted globally by their nearest-grid-row pair and sharded
    contiguously across the 8 cores, so each core only needs a ~26-row band
    of the 128x128 feature grid.  The layer-0 table build (conv3x3 expressed
    as 7 accumulated matmuls per row) therefore runs 8x cheaper than a
    replicated build.
  - The table is stored as row-PAIR slots: slot p holds rows (p-1, p) of the
    band concatenated (512 fp16).  One 1024B dma_gather element fetches both
    y-corners of a query at one x, halving gather descriptor count; the
    transpose gather lands feature-major, exactly the matmul layout.
    Two extra slots hold duplicated edge rows for clamped (0,0)/(127,127)
    pairs.  Rare non-adjacent row pairs (rint ties) are patched on host.
  - Layer-0 combine runs on the tensor engine: an identity matmul accumulates
    the gathered rows into PSUM on top of the K=2 rel-coordinate matmul, so
    the vector engine only does the relu evacuation.
  - All 4 ensemble shifts of a 512-query block are processed per weight load
    (4 matmuls per LDWEIGHTS) into [128,1024] two-bank PSUM tiles; bias+relu
    evacuations alternate between scalar and vector engines.
  - The local-ensemble blend is fused: scalar_tensor_tensor reads layer-4
    PSUM, adds the bias and multiplies the per-query area weight in one op.
"""

import os
import numpy as np

N_CORES = 8
H = W = 128          # feature grid
HP = 130             # padded
Q_TOTAL = 65536
Q_CORE = Q_TOTAL // N_CORES          # 8192
QB = 512                             # query block (matmul free dim)
NQB = Q_CORE // QB                   # 16 blocks per core
NS = 4                               # shifts
D0 = 256                             # layer-0 width
SHIFTS = [(-1, -1), (-1, 1), (1, -1), (1, 1)]

_CACHE = {}


def _build_program(r_rows):
    import concourse.mybir as mybir
    import concourse.tile as tile
    from concourse import bacc

    f16 = mybir.dt.float16
    f32 = mybir.dt.float32
    i16 = mybir.dt.int16

    NSLOT = r_rows + 3               # regular slots 0..r_rows, EL, EH
    EL = r_rows + 1
    EH = r_rows + 2
    FQ = r_rows + 2                  # padded feature rows

    GQ = int(os.environ.get("BASS_KERNEL_GQ", "4"))
    nc = bacc.Bacc("TRN2", target_bir_lowering=False, debug=False,
                   num_devices=N_CORES, num_swdge_queues=GQ)

    featq_d = nc.dram_tensor("featq", [128, FQ * HP], f16, kind="ExternalInput")
    maskq_d = nc.dram_tensor("maskq", [10, FQ * HP], f16, kind="ExternalInput")
    w0gp_d = nc.dram_tensor("w0gp", [128, 3 * D0], f16, kind="ExternalInput")
    w0gs_d = nc.dram_tensor("w0gs", [64, 3 * D0], f16, kind="ExternalInput")
    wx_d = nc.dram_tensor("wx", [10, D0], f16, kind="ExternalInput")
    wq_d = nc.dram_tensor("wq", [2, D0], f16, kind="ExternalInput")
    ident_d = nc.dram_tensor("ident", [128, 128], f16, kind="ExternalInput")
    wl_d = nc.dram_tensor("wl", [128, 12 * 128], f16, kind="ExternalInput")
    w4_d = nc.dram_tensor("w4", [128, 6], f16, kind="ExternalInput")
    bl_d = nc.dram_tensor("bl", [128, 7], f32, kind="ExternalInput")
    idx_d = nc.dram_tensor("idx", [128, NQB * 64], i16, kind="ExternalInput")
    rels_d = nc.dram_tensor("rels", [NQB, 2, NS * QB], f16,
                            kind="ExternalInput")
    wb_d = nc.dram_tensor("wb", [NQB, 3, NS * QB], f16, kind="ExternalInput")
    out_d = nc.dram_tensor("out", [3, Q_CORE], f32, kind="ExternalOutput")

    gsems = [nc.alloc_semaphore(name=f"gsem{i}") for i in range(GQ)]

    with tile.TileContext(nc) as tc:
        with tc.tile_pool(name="constp", bufs=1) as constp, \
             tc.tile_pool(name="gridp", bufs=1) as gridp, \
             tc.tile_pool(name="ztabp", bufs=1) as ztabp, \
             tc.tile_pool(name="zgp", bufs=3) as zgp, \
             tc.tile_pool(name="relp", bufs=3) as relp, \
             tc.tile_pool(name="wbp", bufs=3) as wbp, \
             tc.tile_pool(name="actp", bufs=10) as actp, \
             tc.tile_pool(name="tkp", bufs=3) as tkp, \
             tc.tile_pool(name="pmp", bufs=2) as pmp, \
             tc.tile_pool(name="tmpp", bufs=6) as tmpp, \
             tc.tile_pool(name="outp", bufs=3) as outp, \
             tc.tile_pool(name="pbp", bufs=4, space="PSUM") as pbp:

            wl = constp.tile([128, 12 * 128], f16)
            nc.sync.dma_start(out=wl[:], in_=wl_d[:])
            w4 = constp.tile([128, 6], f16)
            nc.sync.dma_start(out=w4[:], in_=w4_d[:])
            wq = constp.tile([2, D0], f16)
            nc.sync.dma_start(out=wq[:], in_=wq_d[:])
            ident = constp.tile([128, 128], f16)
            nc.sync.dma_start(out=ident[:], in_=ident_d[:])
            bl = constp.tile([128, 7], f32)
            nc.sync.dma_start(out=bl[:], in_=bl_d[:])
            idxs = constp.tile([128, NQB * 64], i16)
            nc.sync.dma_start(out=idxs[:], in_=idx_d[:])

            ztab = ztabp.tile([128, NSLOT, 512], f16, name="ztab")

            # dummy gather on a private table: forces the gpsimd 'mlp' ucode
            # library load during the grid phase instead of before gather 0.
            dztab = constp.tile([128, 1, 512], f16, name="dztab")
            nc.vector.memset(dztab[:], 0)
            dzi = constp.tile([128, 8], i16, name="dzi")
            nc.vector.memset(dzi[:], 0)
            dzo = constp.tile([128, 4, 128], f16, name="dzo")
            nc.gpsimd.dma_gather(
                dzo[:], dztab[:], dzi[:],
                num_idxs=128, num_idxs_reg=128, elem_size=512, transpose=True,
                sbuf_tokens_per_rank=128, sbuf_free_dim_per_rank=1024,
                single_packet=False, queue_num=0)

            # ---------------- grid phase: build layer-0 slot table ----------
            featq = gridp.tile([128, FQ * HP], f16)
            nc.sync.dma_start(out=featq[:], in_=featq_d[:])
            maskq = gridp.tile([10, FQ * HP], f16)
            nc.sync.dma_start(out=maskq[:], in_=maskq_d[:])
            w0gp = gridp.tile([128, 3 * D0], f16)
            nc.sync.dma_start(out=w0gp[:], in_=w0gp_d[:])
            w0gs = gridp.tile([64, 3 * D0], f16)
            nc.sync.dma_start(out=w0gs[:], in_=w0gs_d[:])
            wx = gridp.tile([10, D0], f16)
            nc.sync.dma_start(out=wx[:], in_=wx_d[:])

            ev_ctr = [0]

            def evac_copy(dst, src):
                ev_ctr[0] += 1
                if ev_ctr[0] % 2 == 0:
                    nc.vector.tensor_copy(dst, src)
                else:
                    nc.scalar.copy(dst, src)

            for t in range(r_rows):
                ps = pbp.tile([128, D0], f32, tag="pb")
                for p, off in enumerate((0, HP, 2 * HP)):
                    nc.tensor.matmul(
                        ps[:], featq[:, t * HP + off:t * HP + off + 128],
                        w0gp[:, p * D0:(p + 1) * D0],
                        start=(p == 0), stop=False)
                for s, off in enumerate((2, HP + 2, 2 * HP + 2)):
                    nc.tensor.matmul(
                        ps[:], featq[0:64, t * HP + off:t * HP + off + 128],
                        w0gs[:, s * D0:(s + 1) * D0],
                        start=False, stop=False)
                nc.tensor.matmul(
                    ps[:], maskq[:, t * HP:t * HP + 128], wx[:],
                    start=False, stop=True)
                evac_copy(ztab[:, t, 256:512], ps[:])
                evac_copy(ztab[:, t + 1, 0:256], ps[:])
            # duplicated edge slots: EL=(row0,row0) on core 0, EH=(r127,r127)
            # on core 7 (garbage elsewhere, never indexed)
            evac_copy(ztab[:, EL, 0:256], ztab[:, 1, 256:512])
            evac_copy(ztab[:, EL, 256:512], ztab[:, 1, 256:512])
            evac_copy(ztab[:, EH, 0:256], ztab[:, r_rows - 1, 256:512])
            evac_copy(ztab[:, EH, 256:512], ztab[:, r_rows - 1, 256:512])

            # ---------------- MLP phase ----------------
            def evac_relu(dst, ps, bias_ap):
                ev_ctr[0] += 1
                if ev_ctr[0] % 2 == 0:
                    if bias_ap is None:
                        nc.vector.tensor_scalar_max(dst, ps, 0.0)
                    else:
                        nc.vector.tensor_scalar(
                            out=dst, in0=ps, scalar1=bias_ap, scalar2=0.0,
                            op0=mybir.AluOpType.add, op1=mybir.AluOpType.max)
                else:
                    nc.scalar.activation(
                        dst, ps, mybir.ActivationFunctionType.Relu,
                        bias=0.0 if bias_ap is None else bias_ap)

            for qb in range(NQB):
                zg = zgp.tile([128, 4, 2 * QB], f16, tag="zg")
                g = nc.gpsimd.dma_gather(
                    zg[:], ztab[:, :, :], idxs[:, qb * 64:(qb + 1) * 64],
                    num_idxs=2 * QB, num_idxs_reg=2 * QB,
                    elem_size=512, transpose=True,
                    sbuf_tokens_per_rank=128, sbuf_free_dim_per_rank=1024,
                    single_packet=False, queue_num=(qb + 1) % GQ)
                # Tile's automatic DMASW wait for the gather's consumers lands
                # one gather late, racing the async data landing.  Fence
                # explicitly: the gather's DMA completion bumps gsem; a
                # gpsimd memset carries the wait, and +0 touches on zg give
                # every PE consumer an engine-tick dependency on it.
                qn = (qb + 1) % GQ
                g.then_inc(gsems[qn], 16)
                tk = tkp.tile([1, 4, 1], f16, tag="tk")
                nc.gpsimd.memset(tk[:], 0).wait_op(
                    gsems[qn], 16 * (qb // GQ + 1), "sem-ge")
                nc.vector.tensor_tensor(
                    out=zg[0:1, :, 0:1], in0=zg[0:1, :, 0:1], in1=tk[:],
                    op=mybir.AluOpType.add)
                nc.vector.tensor_tensor(
                    out=zg[0:1, :, QB:QB + 1], in0=zg[0:1, :, QB:QB + 1],
                    in1=tk[:], op=mybir.AluOpType.add)
                rel = relp.tile([2, NS * QB], f16, tag="rel")
                nc.sync.dma_start(out=rel[:], in_=rels_d[qb, :, :])
                wbt = wbp.tile([3, NS * QB], f16, tag="wb")
                nc.sync.dma_start(out=wbt[:], in_=wb_d[qb, :, :])

                # layer 0: identity matmul accumulates gathered rows onto the
                # K=2 rel matmul in PSUM; evac is a pure relu.
                P0 = [[pbp.tile([128, 2 * QB], f32, tag="pb",
                                name=f"p0_{qb}_{_c}_{_y}")
                       for _y in range(2)] for _c in range(2)]
                for c in range(2):
                    for yp in range(2):
                        for xb in range(2):
                            nc.tensor.matmul(
                                P0[c][yp][:, xb * QB:(xb + 1) * QB],
                                ident[:],
                                zg[:, 2 * yp + c, xb * QB:(xb + 1) * QB],
                                start=True, stop=False)
                for c in range(2):
                    for yp in range(2):
                        for xb in range(2):
                            s = 2 * yp + xb
                            nc.tensor.matmul(
                                P0[c][yp][:, xb * QB:(xb + 1) * QB],
                                wq[:, c * 128:(c + 1) * 128],
                                rel[:, s * QB:(s + 1) * QB],
                                start=False, stop=True)
                a = [[actp.tile([128, 2 * QB], f16, tag="a",
                               name=f"a0_{qb}_{_c}_{_y}")
                      for _y in range(2)] for _c in range(2)]
                for c in range(2):
                    for yp in range(2):
                        evac_relu(a[c][yp][:], P0[c][yp][:], None)

                # layers 1..3: shared weights across the 4 shifts
                for l in range(3):
                    Pn = [[pbp.tile([128, 2 * QB], f32, tag="pb",
                                    name=f"pn_{qb}_{l}_{_m}_{_y}")
                           for _y in range(2)] for _m in range(2)]
                    for m in range(2):
                        for k in range(2):
                            blk = ((l * 2 + k) * 2 + m) * 128
                            for yp in range(2):
                                for xb in range(2):
                                    nc.tensor.matmul(
                                        Pn[m][yp][:, xb * QB:(xb + 1) * QB],
                                        wl[:, blk:blk + 128],
                                        a[k][yp][:, xb * QB:(xb + 1) * QB],
                                        start=(k == 0), stop=(k == 1))
                    an = [[actp.tile([128, 2 * QB], f16, tag="a",
                                    name=f"an_{qb}_{l}_{_m}_{_y}")
                           for _y in range(2)] for _m in range(2)]
                    for m in range(2):
                        for yp in range(2):
                            evac_relu(an[m][yp][:], Pn[m][yp][:],
                                      bl[:, 2 * l + m:2 * l + m + 1])
                    a = an

                # layer 4: shifts s0,s1 -> P4a segments, s2,s3 -> P4b
                P4 = [pbp.tile([3, 2 * QB], f32, tag="pb",
                              name=f"p4_{qb}_{_y}") for _y in range(2)]
                for k in range(2):
                    for yp in range(2):
                        for xb in range(2):
                            nc.tensor.matmul(
                                P4[yp][:, xb * QB:(xb + 1) * QB],
                                w4[:, k * 3:(k + 1) * 3],
                                a[k][yp][:, xb * QB:(xb + 1) * QB],
                                start=(k == 0), stop=(k == 1))

                # blend: (P4 + b4) * area_weight, then sum the 4 segments
                pm = pmp.tile([3, 2, 2 * QB], f32, tag="pm")
                for yp in range(2):
                    nc.vector.scalar_tensor_tensor(
                        out=pm[:, yp, :], in0=P4[yp][:],
                        scalar=bl[0:3, 6:7],
                        in1=wbt[:, yp * 2 * QB:(yp + 1) * 2 * QB],
                        op0=mybir.AluOpType.add, op1=mybir.AluOpType.mult)
                t0 = tmpp.tile([3, QB], f32, tag="tmp")
                nc.vector.tensor_tensor(
                    out=t0[:], in0=pm[:, 0, 0:QB], in1=pm[:, 0, QB:2 * QB],
                    op=mybir.AluOpType.add)
                t1 = tmpp.tile([3, QB], f32, tag="tmp")
                nc.vector.tensor_tensor(
                    out=t1[:], in0=pm[:, 1, 0:QB], in1=pm[:, 1, QB:2 * QB],
                    op=mybir.AluOpType.add)
                outt = outp.tile([3, QB], f32, tag="out")
                nc.vector.tensor_tensor(
                    out=outt[:], in0=t0[:], in1=t1[:],
                    op=mybir.AluOpType.add)
                nc.sync.dma_start(
                    out=out_d[:, qb * QB:(qb + 1) * QB], in_=outt[:])

    nc.compile()
    return nc


def _host_prep(masked_feat, mask, hr_coord, Ws, bs, r_rows):
    f32 = np.float32
    FQ = r_rows + 2
    EL = r_rows + 1
    EH = r_rows + 2

    # ---- per-query geometry (mirrors reference in fp32) ----
    coord = hr_coord[0].astype(f32)            # [Q, 2]
    rx = f32(1.0 / H)
    eps = f32(1e-6)
    lo, hi = f32(-1 + 1e-6), f32(1 - 1e-6)
    iy_s, ix_s, rel_s, area_s = [], [], [], []
    for (vx, vy) in SHIFTS:
        cy = np.clip(coord[:, 0] + f32(vx) * rx + eps, lo, hi)
        cx = np.clip(coord[:, 1] + f32(vy) * rx + eps, lo, hi)
        iy = np.clip(np.rint(((cy + f32(1.0)) * f32(H) - f32(1.0)) * f32(0.5)
                             ).astype(np.int32), 0, H - 1)
        ix = np.clip(np.rint(((cx + f32(1.0)) * f32(W) - f32(1.0)) * f32(0.5)
                             ).astype(np.int32), 0, W - 1)
        iy_s.append(iy)
        ix_s.append(ix)
        qy = f32(-1.0) + (f32(2.0) * iy.astype(f32) + f32(1.0)) / f32(H)
        qx = f32(-1.0) + (f32(2.0) * ix.astype(f32) + f32(1.0)) / f32(W)
        r0 = (coord[:, 0] - qy) * f32(H)
        r1 = (coord[:, 1] - qx) * f32(W)
        rel_s.append(np.stack([r0, r1]))       # [2, Q]
        area_s.append(np.abs(r0 * r1) + f32(1e-9))
    tot = area_s[0] + area_s[1] + area_s[2] + area_s[3]
    wgt = [area_s[3 - s] / tot for s in range(NS)]   # local-ensemble swap

    iy0, iy1 = iy_s[0], iy_s[2]
    ix0, ix1 = ix_s[0], ix_s[1]
    Q = coord.shape[0]
    jy = np.zeros(Q, np.int32)
    anom = np.zeros(Q, bool)
    both0 = (iy0 == 0) & (iy1 == 0)
    both127 = (iy0 == 127) & (iy1 == 127)
    normal = (iy1 == iy0 + 1)
    jy[normal] = iy0[normal] + 1
    jy[both0] = 0
    jy[both127] = 128
    rest = ~(normal | both0 | both127)
    anom[rest] = True
    jy[rest] = np.clip(iy0[rest] + 1, 1, 127)

    order = np.argsort(jy, kind="stable")

    # needed row span per core (for adaptive recompiles)
    needed = 0
    rbases = []
    for c in range(N_CORES):
        jys = jy[order[c * Q_CORE:(c + 1) * Q_CORE]]
        jlo, jhi = int(jys.min()), int(jys.max())
        if c == 0:
            rbase = -1
        elif c == N_CORES - 1:
            rbase = H - r_rows
        else:
            rbase = jlo - 1
        rbases.append(rbase)
        reg = (jys != 0) & (jys != 128)
        if reg.any():
            pmax = int((jys[reg] - rbase).max())
            pmin = int((jys[reg] - rbase).min())
            needed = max(needed, pmax + 1, pmax + 1 + max(0, 1 - pmin))
        if c == N_CORES - 1 and reg.any():
            needed = max(needed, H - int(jys[reg].min()) + 2)
    if needed > r_rows:
        return None, needed, None, None   # caller recompiles

    # ---- global weight packing (identical across cores) ----
    W0 = Ws[0].astype(f32)
    cc = np.arange(64) * 9
    w0gp = np.zeros((128, 3 * D0), dtype=np.float16)
    for p, (oa, ob) in enumerate(((0, 1), (3, 4), (6, 7))):
        w0gp[0:64, p * D0:(p + 1) * D0] = W0[cc + oa, :]
        w0gp[64:128, p * D0:(p + 1) * D0] = W0[cc + ob, :]
    w0gs = np.zeros((64, 3 * D0), dtype=np.float16)
    for s, o in enumerate((2, 5, 8)):
        w0gs[:, s * D0:(s + 1) * D0] = W0[cc + o, :]
    wx = np.zeros((10, D0), dtype=np.float16)
    for o in range(9):
        wx[o] = W0[64 * 9 + o, :]
    wx[9] = bs[0].astype(f32)
    wq = W0[585:587, :].astype(np.float16)
    ident = np.eye(128, dtype=np.float16)

    wl = np.zeros((128, 12 * 128), dtype=np.float16)
    for l in range(3):
        Wl = Ws[l + 1].astype(f32)
        for k in range(2):
            for m in range(2):
                blk = ((l * 2 + k) * 2 + m) * 128
                wl[:, blk:blk + 128] = Wl[k * 128:(k + 1) * 128,
                                          m * 128:(m + 1) * 128]
    w4 = np.zeros((128, 6), dtype=np.float16)
    w4[:, 0:3] = Ws[4][0:128].astype(f32)
    w4[:, 3:6] = Ws[4][128:256].astype(f32)
    bl = np.zeros((128, 7), dtype=f32)
    for l in range(3):
        for m in range(2):
            bl[:, 2 * l + m] = bs[l + 1][m * 128:(m + 1) * 128]
    bl[0:3, 6] = bs[4]

    # padded feature image (global, then sliced per core)
    ch = np.zeros((64, HP, HP), dtype=f32)
    ch[:, 1:129, 1:129] = masked_feat[0]
    chf = ch.reshape(64, -1).astype(np.float16)        # [64, HP*HP]
    mpad = np.zeros((HP, HP), dtype=f32)
    mpad[1:129, 1:129] = mask[0, 0]
    mflat = mpad.reshape(-1).astype(np.float16)

    def padslice(src2d, start_row, n_rows):
        # rows of the padded image, zero-padded outside [0, HP)
        n_ch = src2d.shape[0]
        out = np.zeros((n_ch, n_rows * HP), dtype=np.float16)
        a = max(0, start_row)
        b = min(HP, start_row + n_rows)
        if a < b:
            out[:, (a - start_row) * HP:(b - start_row) * HP] = \
                src2d[:, a * HP:b * HP]
        return out

    in_maps, gidxs = [], []
    anom_info = []
    for c in range(N_CORES):
        gidx = order[c * Q_CORE:(c + 1) * Q_CORE]
        gidxs.append(gidx)
        rbase = rbases[c]
        jys = jy[gidx]
        slot = np.where(jys == 0, EL,
                        np.where(jys == 128, EH, jys - rbase)).astype(np.int64)
        # window t covers padded rows rbase+t .. rbase+t+2
        featq = np.zeros((128, FQ * HP), dtype=np.float16)
        fq0 = padslice(chf, rbase, FQ)
        featq[0:64] = fq0
        featq[64:128, 0:FQ * HP - 1] = fq0[:, 1:]
        maskq = np.zeros((10, FQ * HP), dtype=np.float16)
        for o in range(9):
            ki, kj = o // 3, o % 3
            off = ki * HP + kj
            # maskq[o] at local pos z = mflat[(rbase+?)... shifted]
            src = np.zeros(FQ * HP, dtype=np.float16)
            g0 = rbase * HP + off
            a = max(0, g0)
            b = min(HP * HP, g0 + FQ * HP)
            if a < b:
                src[a - g0:b - g0] = mflat[a:b]
            maskq[o] = src
        maskq[9, :] = 1.0

        idxw = np.zeros((128, NQB * 64), dtype=np.int16)
        rels = np.zeros((NQB, 2, NS * QB), dtype=np.float16)
        wb = np.zeros((NQB, 3, NS * QB), dtype=np.float16)
        for qb in range(NQB):
            qsl = gidx[qb * QB:(qb + 1) * QB]
            ssl = slot[qb * QB:(qb + 1) * QB]
            ents = np.concatenate([ssl * 128 + ix0[qsl],
                                   ssl * 128 + ix1[qsl]]).astype(np.int16)
            idxw[:, qb * 64:(qb + 1) * 64] = np.tile(
                ents.reshape(64, 16).T, (8, 1))
            for s in range(NS):
                rels[qb, :, s * QB:(s + 1) * QB] = \
                    rel_s[s][:, qsl].astype(np.float16)
                wv = wgt[s][qsl].astype(np.float16)
                if s >= 2:
                    wv = np.where(anom[qsl], np.float16(0), wv)
                wb[qb, :, s * QB:(s + 1) * QB] = wv[None, :]
        in_maps.append({
            "featq": featq, "maskq": maskq, "w0gp": w0gp, "w0gs": w0gs,
            "wx": wx, "wq": wq, "ident": ident, "wl": wl, "w4": w4,
            "bl": bl, "idx": idxw, "rels": rels, "wb": wb,
        })

    # anomaly info for host-side correction of shifts s2, s3
    aq = np.nonzero(anom)[0]
    if len(aq):
        anom_info = dict(q=aq, iy1=iy1[aq], ix0=ix0[aq], ix1=ix1[aq],
                         rel2=rel_s[2][:, aq], rel3=rel_s[3][:, aq],
                         w2=wgt[2][aq], w3=wgt[3][aq])
    else:
        anom_info = None
    return in_maps, needed, gidxs, anom_info


def _host_fix(out_full, anom_info, masked_feat, mask, Ws, bs):
    """Patch shifts s2/s3 of queries whose y-corner pair is non-adjacent."""
    if anom_info is None:
        return
    f32 = np.float32
    feat = np.concatenate([masked_feat, mask], axis=1)[0].astype(f32)
    p = np.pad(feat, ((0, 0), (1, 1), (1, 1)))
    W0 = Ws[0].astype(f32)
    for n, q in enumerate(anom_info['q']):
        iyv = int(anom_info['iy1'][n])
        for s, ixk, relk, wk in ((2, 'ix0', 'rel2', 'w2'),
                                 (3, 'ix1', 'rel3', 'w3')):
            ixv = int(anom_info[ixk][n])
            patch = p[:, iyv:iyv + 3, ixv:ixv + 3]          # [65, 3, 3]
            qf = patch.transpose(0, 1, 2).reshape(65 * 9)   # (c, ki, kj)
            rel = anom_info[relk][:, n]
            x = W0[:585].T @ qf + W0[585:587].T @ rel + bs[0]
            x = np.maximum(x, 0)
            for li in range(1, 4):
                x = np.maximum(Ws[li].T @ x + bs[li], 0)
            pred = Ws[4].T @ x + bs[4]
            out_full[:, q] += pred.astype(f32) * f32(anom_info[wk][n])


def kernel(masked_feat, mask, gt_feat, hr_coord,
           W0, b0, W1, b1, W2, b2, W3, b3, W4, b4):
    from concourse.bass_utils import run_bass_kernel_spmd

    masked_feat = np.asarray(masked_feat)
    mask = np.asarray(mask)
    hr_coord = np.asarray(hr_coord)
    Ws = [np.asarray(w) for w in (W0, W1, W2, W3, W4)]
    bs = [np.asarray(b) for b in (b0, b1, b2, b3, b4)]

    r_rows = _CACHE.get("r_rows", int(os.environ.get("BASS_KERNEL_R", "26")))
    in_maps, needed, gidxs, anom_info = _host_prep(
        masked_feat, mask, hr_coord, Ws, bs, r_rows)
    if in_maps is None:
        r_rows = needed + 2
        in_maps, needed, gidxs, anom_info = _host_prep(
            masked_feat, mask, hr_coord, Ws, bs, r_rows)
        assert in_maps is not None
    if _CACHE.get("r_rows") != r_rows:
        _CACHE["nc"] = _build_program(r_rows)
        _CACHE["r_rows"] = r_rows
    nc = _CACHE["nc"]

    trace = bool(int(os.environ.get("BASS_KERNEL_TRACE", "0")))
    res = run_bass_kernel_spmd(nc, in_maps, list(range(N_CORES)), trace=trace)
    _CACHE["last_results"] = res

    full = np.zeros((3, Q_TOTAL), dtype=np.float32)
    for c in range(N_CORES):
        full[:, gidxs[c]] = res.results[c]["out"]
    _host_fix(full, anom_info, masked_feat, mask, Ws, bs)
    return full.reshape(1, 3, 256, 256).astype(np.float32)
